# revision 1
# baseline (speedup 1.0000x reference)
"""ConvexSoftMixer Trainium2 kernel.

Shards batch*heads (1*8 = 8) across 8 NeuronCores, one head per core.

Math (exact refactor of the reference; m1 cancels analytically):
    f_q[s] = sum_j softplus(softplus(q @ spW1q.T + b1) @ spW2q.T + b2)[s,j]
    g_k[t] likewise for k
    phi_q = exp(q @ Wh.T); phi_k = exp(k @ Wh.T); u = v @ Wv.T
    c[t,p]  = g_k[t] - log(S) + u[t,p]
    m2[p]   = max_t c[t,p]
    E[t,p]  = exp(c[t,p] - m2[p])
    M[r,p]  = sum_t phi_k[t,r] * E[t,p]
    y[s,p]  = f_q[s] + m2[p] + log( sum_r phi_q[s,r] * M[r,p] )
(The -log(S) is folded into g_k: it shifts m2 by -log(S) and cancels in E.)

On-device layout is transposed (feature dim on SBUF partitions, sequence on
the free dim) so the ICNN layers chain as matmuls with no transposes. q and
k ICNNs are stacked on 128 partitions with block-diagonal weights. All
partition-dim broadcasts are done as rank-1 matmul accumulations into PSUM
using constant rows packed into the host-prepared input tensors.
"""

import math

import numpy as np

_B, _H, _S, _D, _P = 1, 8, 512, 64, 32
_NCORES = 8
_LN_S = math.log(float(_S))

_CACHE = {}


def _build_bass(dump=False):
    import concourse.tile as tile
    from concourse import bacc, mybir

    f32 = mybir.dt.float32
    AF = mybir.ActivationFunctionType
    AX = mybir.AxisListType.X

    # Bacc (not raw Bass): its compile passes split multi-sem waits (TRN2
    # allows one wait per instruction) and insert ACT table loads.
    nc = bacc.Bacc("TRN2", target_bir_lowering=False, debug=False)

    # DRAM I/O (per core). Read-only inputs ride in ONE tensor/DMA; column map:
    # [0:512) xqk | [512:643) w1b | [643:772) w2b | [772:836) whv (rows 0-65)
    # | [836:1348) kt (rows 0-63).  vta is separate because the device writes
    # g_k into its row 64 (tile-granular deps stay exact that way).
    _MW = 1348
    mega_d = nc.dram_tensor("mega", [128, _MW], f32, kind="ExternalInput").ap()
    vta_d = nc.dram_tensor("vta", [_D + 2, _S], f32, kind="ExternalInput").ap()
    misc_d = nc.dram_tensor("misc", [1, 128 + _S], f32, kind="ExternalInput").ap()
    y_d = nc.dram_tensor("y", [_P, _S], f32, kind="ExternalOutput").ap()

    NCH = _S // 128  # 4 sequence chunks of 128 for [t, p]-layout stages

    with tile.TileContext(nc) as tc:
        with (
            tc.tile_pool(name="pin", bufs=1) as pin,
            tc.tile_pool(name="pwork", bufs=1) as pw,
            # PSUM: tags share slots; lifetimes are disjoint within a tag.
            tc.tile_pool(name="psA", bufs=2, space="PSUM") as psA,  # z1,z2 / AT,F
            tc.tile_pool(name="psB", bufs=2, space="PSUM") as psB,  # gk,cT / phiq,M
            tc.tile_pool(name="psC", bufs=2, space="PSUM") as psC,  # pk, ec
            tc.tile_pool(name="psD", bufs=1, space="PSUM") as psD,  # fq
        ):
            # ---- input loads ----
            mega = pin.tile([128, _MW], f32, tag="mega")
            nc.sync.dma_start(out=mega, in_=mega_d)
            vta = pin.tile([_D + 2, _S], f32, tag="vta")
            nc.sync.dma_start(out=vta, in_=vta_d)
            misc = pin.tile([1, 128 + _S], f32, tag="misc")
            nc.sync.dma_start(out=misc, in_=misc_d)

            xqk = mega[:, 0:512]
            w1b = mega[:, 512:643]
            w2b = mega[:, 643:772]
            whv = mega[0:_D + 2, 772:836]
            kt = mega[0:_D, 836:1348]

            # named slices of the packed inputs
            w1 = w1b[:, 0:128]        # block-diag softplus'd layer-1 weights (T)
            b1 = w1b[:, 128:129]      # stacked layer-1 bias column
            eq = w1b[:, 129:130]      # [1]*64 + [0]*64 column
            ek = w1b[:, 130:131]      # [0]*64 + [1]*64 column
            w2 = w2b[:, 0:128]
            wv_aug = whv[:, 0:_P]     # rows 0-63 Wv.T, row 64 = 1.0, row 65 = 0
            wh_t = whv[0:_D, _P:2 * _P]  # Wh.T
            b2row = misc[0:1, 0:128]  # layer-2 bias as a [1, 128] row
            ones_row = misc[0:1, 128:128 + _S]  # [1, S] of 1.0

            # ---- stacked ICNN (q rows 0-63, k rows 64-127) ----
            z1_p = psA.tile([128, _S], f32, tag="big")
            nc.tensor.matmul(out=z1_p, lhsT=w1, rhs=xqk, start=True, stop=True)
            e1 = pw.tile([128, _S], f32, tag="e1")
            nc.scalar.activation(out=e1, in_=z1_p, func=AF.Exp, bias=b1, scale=1.0)
            z1 = pw.tile([128, _S], f32, tag="z1")
            nc.scalar.activation(out=z1, in_=e1, func=AF.Ln, bias=1.0, scale=1.0)

            # layer-2 args can exceed the Exp LUT's input clamp (~41), so:
            # softplus(x) = max(x, ln(1 + exp(min(x, 30))))   (exact in f32:
            # for x > 30, softplus(x) == x and softplus >= x always).
            z2_p = psA.tile([128, _S], f32, tag="big")
            nc.tensor.matmul(out=z2_p, lhsT=w2, rhs=z1, start=True, stop=False)
            nc.tensor.matmul(out=z2_p, lhsT=b2row, rhs=ones_row,
                             start=False, stop=True)  # + b2 broadcast
            z2c = pw.tile([128, _S], f32, tag="z2c")
            nc.vector.tensor_scalar_min(z2c, z2_p, 30.0)
            e2 = pw.tile([128, _S], f32, tag="e2")
            nc.scalar.activation(out=e2, in_=z2c, func=AF.Exp, bias=0.0, scale=1.0)
            l2 = pw.tile([128, _S], f32, tag="l2")
            nc.scalar.activation(out=l2, in_=e2, func=AF.Ln, bias=1.0, scale=1.0)
            z2 = pw.tile([128, _S], f32, tag="z2")
            nc.vector.tensor_max(out=z2, in0=l2, in1=z2_p)

            # ---- phi_k chunks [t,r] (independent of ICNN; overlaps) ----
            pk_p = psC.tile([128, NCH * _P], f32, tag="chunk")
            for c in range(NCH):
                nc.tensor.matmul(
                    out=pk_p[:, c * _P:(c + 1) * _P],
                    lhsT=kt[:, c * 128:(c + 1) * 128],
                    rhs=wh_t,
                    start=True, stop=True,
                )
            pk = pw.tile([128, NCH * _P], f32, tag="pk")
            nc.scalar.activation(out=pk, in_=pk_p, func=AF.Exp, bias=0.0, scale=1.0)

            # ---- phi_q [r, s] ----
            phiq_p = psB.tile([_P, _S], f32, tag="mid")
            nc.tensor.matmul(out=phiq_p, lhsT=wh_t, rhs=xqk[0:_D, :], start=True, stop=True)
            phiq = pw.tile([_P, _S], f32, tag="phiq")
            nc.scalar.activation(out=phiq, in_=phiq_p, func=AF.Exp, bias=0.0, scale=1.0)

            # ---- f_q, g_k row sums of z2 (masked ones matmuls) ----
            fq_p = psD.tile([1, _S], f32, tag="fq")
            nc.tensor.matmul(out=fq_p, lhsT=eq, rhs=z2, start=True, stop=True)
            gk_p = psB.tile([1, _S], f32, tag="mid")
            nc.tensor.matmul(out=gk_p, lhsT=ek, rhs=z2, start=True, stop=True)

            fq = pw.tile([1, _S], f32, tag="fq_sb")
            nc.vector.tensor_copy(out=fq, in_=fq_p)
            # g_k - log(S) written into vta row 64 (pairs with wv_aug's 1.0 row)
            nc.vector.tensor_scalar_add(vta[_D:_D + 1, :], gk_p, -_LN_S)

            # ---- cT[p,t] = u.T + g_k broadcast ; m2 = rowmax ----
            cT_p = psB.tile([_P, _S], f32, tag="mid")
            nc.tensor.matmul(
                out=cT_p, lhsT=wv_aug[0:_D + 1, :], rhs=vta[0:_D + 1, :],
                start=True, stop=True,
            )
            m2pad = pw.tile([_P, _P], f32, tag="m2pad")
            nc.vector.memset(m2pad, 0.0)
            nc.vector.reduce_max(m2pad[:, 0:1], cT_p, axis=AX)
            m2t = pw.tile([_P, _P], f32, tag="m2t")
            nc.vector.transpose(m2t, m2pad)  # row 0 of m2t = m2 as [1, P]
            negm2_4 = pw.tile([1, NCH * _P], f32, tag="negm2")
            for c in range(NCH):
                nc.vector.tensor_scalar_mul(
                    negm2_4[0:1, c * _P:(c + 1) * _P], m2t[0:1, 0:_P], -1.0)

            # ---- E chunks [t,p] = exp(u + g_k - m2) ----
            # u + g_k via the augmented matmul (vta rows 64=g_k, 65=1.0 paired
            # with wv_aug rows 64=1.0, 65=0), then one rank-1 matmul adds the
            # tiled -m2 row across all four chunks at once.
            ec_p = psC.tile([128, NCH * _P], f32, tag="chunk")
            for c in range(NCH):
                nc.tensor.matmul(
                    out=ec_p[:, c * _P:(c + 1) * _P],
                    lhsT=vta[:, c * 128:(c + 1) * 128],
                    rhs=wv_aug,
                    start=True, stop=False,
                )
                nc.tensor.matmul(
                    out=ec_p[:, c * _P:(c + 1) * _P],
                    lhsT=ones_row[0:1, c * 128:(c + 1) * 128],
                    rhs=negm2_4[0:1, 0:_P],
                    start=False, stop=True,
                )
            ec = pw.tile([128, NCH * _P], f32, tag="ec")
            nc.scalar.activation(out=ec, in_=ec_p, func=AF.Exp, bias=0.0, scale=1.0)

            # ---- M[r,p] = sum_t phi_k E ----
            M_p = psB.tile([_P, _P], f32, tag="mid")
            for c in range(NCH):
                nc.tensor.matmul(
                    out=M_p,
                    lhsT=pk[:, c * _P:(c + 1) * _P],
                    rhs=ec[:, c * _P:(c + 1) * _P],
                    start=(c == 0), stop=(c == NCH - 1),
                )
            M_sb = pw.tile([_P, _P], f32, tag="M_sb")
            nc.vector.tensor_copy(out=M_sb, in_=M_p)

            # ---- A.T = M.T-style matmul; y ----
            at_p = psA.tile([_P, _S], f32, tag="big")
            nc.tensor.matmul(out=at_p, lhsT=M_sb, rhs=phiq, start=True, stop=True)

            # F[p,s] = f_q[s] + m2[p] (two rank-1 broadcasts)
            f_p = psA.tile([_P, _S], f32, tag="big")
            nc.tensor.matmul(out=f_p, lhsT=ones_row[0:1, 0:_P], rhs=fq,
                             start=True, stop=False)
            nc.tensor.matmul(out=f_p, lhsT=m2t[0:1, 0:_P], rhs=ones_row,
                             start=False, stop=True)

            lnA = pw.tile([_P, _S], f32, tag="lnA")
            nc.scalar.activation(out=lnA, in_=at_p, func=AF.Ln, bias=0.0, scale=1.0)
            yT = pw.tile([_P, _S], f32, tag="yT")
            nc.vector.tensor_add(out=yT, in0=lnA, in1=f_p)

            nc.sync.dma_start(out=y_d, in_=yT)

            if dump:
                for nm, t in [
                    ("d_z1", z1), ("d_z2", z2), ("d_fq", fq), ("d_pk", pk),
                    ("d_ec", ec), ("d_phiq", phiq), ("d_m2t", m2t),
                    ("d_Msb", M_sb), ("d_lnA", lnA), ("d_vta64", vta[_D:_D + 1, :]),
                    ("d_negm2", negm2_4[0:1, 0:_P]),
                ]:
                    dd = nc.dram_tensor(nm, list(t.shape), f32,
                                        kind="ExternalOutput").ap()
                    nc.sync.dma_start(out=dd, in_=t)

    if not nc.is_finalized():
        nc.finalize()  # runs Bacc passes (wait splitting, reg alloc, ACT table loads)
    return nc


def _host_inputs(q, k, v, spW1q, b1q, spW2q, b2q, spW1k, b1k, spW2k, b2k, Wh, Wv):
    """Build the per-core input maps (numpy layout prep only)."""
    S, D, P = _S, _D, _P
    z = np.zeros
    # block-diagonal transposed weights + packed bias/mask columns (shared)
    w1b = z((128, 131), np.float32)
    w1b[0:D, 0:D] = spW1q.T
    w1b[D:2 * D, D:2 * D] = spW1k.T
    w1b[0:D, 128] = b1q
    w1b[D:2 * D, 128] = b1k
    w1b[0:D, 129] = 1.0     # eq
    w1b[D:2 * D, 130] = 1.0  # ek
    w2b = z((128, 129), np.float32)
    w2b[0:D, 0:D] = spW2q.T
    w2b[D:2 * D, D:2 * D] = spW2k.T
    w2b[0:D, 128] = b2q
    w2b[D:2 * D, 128] = b2k
    whv = z((D + 2, 2 * P), np.float32)
    whv[0:D, 0:P] = Wv.T
    whv[D, 0:P] = 1.0       # pairs with the g_k row of vta
    whv[0:D, P:2 * P] = Wh.T
    misc = z((1, 128 + S), np.float32)
    misc[0, 0:D] = b2q
    misc[0, D:128] = b2k
    misc[0, 128:] = 1.0

    in_maps = []
    for h in range(_H):
        qT = np.ascontiguousarray(q[0, h].T)
        kT = np.ascontiguousarray(k[0, h].T)
        vT = v[0, h].T
        mega = z((128, 1348), np.float32)
        mega[0:D, 0:S] = qT
        mega[D:2 * D, 0:S] = kT
        mega[:, 512:643] = w1b
        mega[:, 643:772] = w2b
        mega[0:D + 2, 772:836] = whv
        mega[0:D, 836:1348] = kT
        vta = z((D + 2, S), np.float32)
        vta[0:D] = vT
        # row D gets g_k - log(S) on device; row D+1 is constant ones
        vta[D + 1] = 1.0
        in_maps.append(dict(mega=mega, vta=vta, misc=misc))
    return in_maps


def kernel(**inputs):
    from concourse.bass_utils import run_bass_kernel_spmd

    np_in = {k: np.asarray(v) for k, v in inputs.items()}
    q, k, v = np_in["q"], np_in["k"], np_in["v"]

    def sp(x):  # softplus for the small weight matrices (host prep)
        return np.log1p(np.exp(x.astype(np.float64))).astype(np.float32)

    in_maps = _host_inputs(
        q, k, v,
        sp(np_in["sq_raw1"]), np_in["sq_b1"], sp(np_in["sq_raw2"]), np_in["sq_b2"],
        sp(np_in["sk_raw1"]), np_in["sk_b1"], sp(np_in["sk_raw2"]), np_in["sk_b2"],
        np_in["Wh"], np_in["Wv"],
    )

    if "nc" not in _CACHE:
        _CACHE["nc"] = _build_bass()
    nc = _CACHE["nc"]

    res = run_bass_kernel_spmd(nc, in_maps, list(range(_NCORES)))
    out = np.zeros((_B, _H, _S, _P), np.float32)
    for h in range(_H):
        out[0, h] = res.results[h]["y"].T
    return out



# revision 11
# speedup vs baseline: 2.1750x; 2.1750x over previous
"""ConvexSoftMixer Trainium2 kernel (v2: bf16 matmuls, single act table).

Shards batch*heads (1*8 = 8) across 8 NeuronCores, one head per core.

Math (exact refactor of the reference; both max-shifts replaced by one
per-head host-side stability constant GKOFF >= max_t g_k, which cancels
analytically):
    f_q[s] = sum_j softplus(softplus(q @ spW1q.T) @ spW2q.T)[s,j]
    g_k[t] likewise for k
    E[t,p]  = exp(u[t,p] - GKOFF + g_k[t]),  u = v @ Wv.T
    M[r,p]  = sum_t exp(k@Wh.T)[t,r] * E[t,p]
    y[s,p]  = f_q[s] + GKOFF - log(S) + log( sum_r exp(q@Wh.T)[s,r] * M[r,p] )

Device layout is transposed (feature dim on SBUF partitions, sequence on
the free dim).  The two ICNNs are stacked on 128 partitions (k rows 0:63,
q rows 64:127) with block-diagonal weights.  All partition-dim broadcasts
ride as extra rank-1 rows inside existing matmul accumulation groups, so
there are no standalone broadcast matmuls and no on-device reductions.

Softplus has no LUT on this toolchain (gen3 act tables); layer 1 uses
ln(1+e^x) directly (args within ~|6|), layer 2 uses the overflow-safe
x + ln(1+e^{-x}) (args reach ~53, beyond the Exp LUT's upper clamp, but
e^{-x} only needs the benign negative domain).  Only Exp and Ln are used,
both served by act table 6 (natural_log_exp_and_others); one manual
InstLoadActFuncSet(6) up front satisfies the table-load analysis, so no
per-switch table reloads are inserted (the baseline lost ~7.7us to 6).

All matmul operands are bf16 (1 PE cycle/row vs 4 for fp32); accumulation
stays fp32 in PSUM.  Numpy emulation of this exact dtype flow gives max
abs err ~7 on y (rel ~1.1e-3 against the harness 2e-2 gate).
"""

import math

import numpy as np

_B, _H, _S, _D, _P = 1, 8, 512, 64, 32
_NCORES = 8
_LN_S = math.log(float(_S))

# mega1 column map (all bf16, [128, _MW1]) -- z1 operands, shipped first
_XKQ = 0       # 512 cols: rows 0:63 = k^T, rows 64:127 = q^T
_W1 = 512      # 128 cols: block-diag layer-1 weights (k TL, q BR)
_MW1 = 640
# mega2 column map (all bf16, [128, _MW2]) -- everything else
_VA = 0        # 512 cols: rows 0:63 = v^T, rows 64:66 = 1.0
_W2 = 512      # 128 cols: block-diag layer-2 weights
_WH0 = 640     # 32 cols: rows 0:63 = Wh.T        (pk rhs)
_WH64 = 672    # 32 cols: rows 64:127 = Wh.T      (phiq lhsT)
_WVA = 704     # 32 cols: rows 0:63 = Wv.T, row 64 = -GKOFF
_EK1 = 736     # 32 cols: rows 0:63 = 1.0         (Eg l2-sum rhs)
_EQ1 = 768     # 32 cols: rows 64:127 = 1.0       (F l2-sum lhsT)
_CF2 = 800     # 32 cols: row 64 = GKOFF, row 65 = -ln S
_WKS = 832     # 32 cols: each col = rowsums of W2's k block   (U a2-sum rhs)
_WQS = 864     # 32 cols: each col = rowsums of W2's q block   (F a2-sum lhsT)
_MW2 = 896

_CACHE = {}


def _build_bass(dump=False):
    import concourse.tile as tile
    from concourse import bacc, mybir

    f32 = mybir.dt.float32
    bf16 = mybir.dt.bfloat16
    AF = mybir.ActivationFunctionType

    nc = bacc.Bacc("TRN2", target_bir_lowering=False, debug=False)

    mega1_d = nc.dram_tensor("mega1", [128, _MW1], bf16, kind="ExternalInput").ap()
    mega2_d = nc.dram_tensor("mega2", [128, _MW2], bf16, kind="ExternalInput").ap()
    y_d = nc.dram_tensor("y", [_P, _S], f32, kind="ExternalOutput").ap()

    NCH = _S // 128  # 4 sequence chunks of 128 for [t, *] stages

    with tile.TileContext(nc) as tc:
        with (
            tc.tile_pool(name="pin", bufs=1) as pin,
            tc.tile_pool(name="pw", bufs=1) as pw,
            # PSUM is bank-granular (8 x 2KB); ring-share tags keep us at 8.
            tc.tile_pool(name="psBig", bufs=2, space="PSUM") as psBig,  # z1,z2,at
            tc.tile_pool(name="psMid", bufs=2, space="PSUM") as psMid,  # phiq,F
            tc.tile_pool(name="psSm", bufs=2, space="PSUM") as psSm,    # pk,M
            tc.tile_pool(name="psU", bufs=1, space="PSUM") as psU,      # U (u part)
            tc.tile_pool(name="psE", bufs=1, space="PSUM") as psE,      # E (gk part)
        ):
            # one act table serves every Exp/Ln below; loading it up front
            # (overlapping the input DMA) suppresses all auto-inserted loads
            nc.scalar.add_instruction(mybir.InstLoadActFuncSet(
                name=nc.get_next_instruction_name(), act_func_set_id=6))

            mega1 = pin.tile([128, _MW1], bf16, tag="mega1")
            nc.sync.dma_start(out=mega1, in_=mega1_d)
            mega2 = pin.tile([128, _MW2], bf16, tag="mega2")
            nc.sync.dma_start(out=mega2, in_=mega2_d)

            xkq = mega1[:, _XKQ:_XKQ + _S]
            kt = mega1[0:_D, _XKQ:_XKQ + _S]
            qt = mega1[_D:128, _XKQ:_XKQ + _S]
            w1 = mega1[:, _W1:_W1 + 128]
            va = mega2[:, _VA:_VA + _S]
            w2 = mega2[:, _W2:_W2 + 128]
            wh0 = mega2[0:_D, _WH0:_WH0 + _P]
            wh64 = mega2[_D:128, _WH64:_WH64 + _P]
            wva = mega2[0:_D + 1, _WVA:_WVA + _P]
            ek1 = mega2[:, _EK1:_EK1 + _P]
            eq1 = mega2[:, _EQ1:_EQ1 + _P]
            cf2 = mega2[_D:_D + 2, _CF2:_CF2 + _P]
            wks = mega2[:, _WKS:_WKS + _P]
            wqs = mega2[:, _WQS:_WQS + _P]

            # ---- PE: stacked ICNN layer 1 (k rows 0:63, q rows 64:127) ----
            z1_p = psBig.tile([128, _S], f32, tag="big")
            nc.tensor.matmul(out=z1_p, lhsT=w1, rhs=xkq, start=True, stop=True)

            # ---- PE: independent early work (fills ACT softplus latency) ----
            phiq_p = psMid.tile([_P, _S], f32, tag="mid")
            nc.tensor.matmul(out=phiq_p, lhsT=wh64, rhs=qt, start=True, stop=True)
            pk_p = psSm.tile([128, NCH * _P], f32, tag="sm")
            for c in range(NCH):
                nc.tensor.matmul(
                    out=pk_p[:, c * _P:(c + 1) * _P],
                    lhsT=kt[:, c * 128:(c + 1) * 128],
                    rhs=wh0,
                    start=True, stop=True,
                )
            # E part 1: U[t,p] = u[t,p] - GKOFF  (v rows + ones row).  Each
            # chunk's group stays open; it is closed after z1 lands by the
            # a2-column-sum matmul (sum_j<64 a2 = W2-k-block-rowsums . z1),
            # so g_k's a2 half never needs z2 materialized.  One group may
            # be pending per PSUM tile, so only chunk 0 opens early.
            U_p = psU.tile([128, NCH * _P], f32, tag="U")
            nc.tensor.matmul(out=U_p[:, 0:_P], lhsT=va[0:_D + 1, 0:128],
                             rhs=wva, start=True, stop=False)

            # ---- ACT: softplus layer 1 = Ln(Exp(a1) + 1)  (args |a1| < ~6) ----
            e1 = pw.tile([128, _S], f32, tag="e1")
            nc.scalar.activation(out=e1, in_=z1_p, func=AF.Exp)
            z1 = pw.tile([128, _S], bf16, tag="z1")
            nc.scalar.activation(out=z1, in_=e1, func=AF.Ln, bias=1.0)

            # ---- PE: layer 2 matmul ----
            z2_p = psBig.tile([128, _S], f32, tag="big")
            nc.tensor.matmul(out=z2_p, lhsT=w2, rhs=z1, start=True, stop=True)

            # ---- PE: finish U (runs in the e2m/l2 shadow) ----
            for c in range(NCH):
                if c > 0:
                    nc.tensor.matmul(
                        out=U_p[:, c * _P:(c + 1) * _P],
                        lhsT=va[0:_D + 1, c * 128:(c + 1) * 128],
                        rhs=wva, start=True, stop=False,
                    )
                nc.tensor.matmul(
                    out=U_p[:, c * _P:(c + 1) * _P],
                    lhsT=z1[:, c * 128:(c + 1) * 128],
                    rhs=wks, start=False, stop=True,
                )

            # ---- ACT: pk exp (overlaps the layer-2 softplus region) ----
            pk = pw.tile([128, NCH * _P], bf16, tag="pk")
            nc.scalar.activation(out=pk, in_=pk_p, func=AF.Exp)

            # ---- ACT: softplus layer 2's Ln half: l2 = Ln(Exp(-a2) + 1) ----
            # (a2 in [0.07, ~53]; e^{-a2} underflows against 1.0 for large a2.
            #  z2 = a2 + l2 itself is never materialized: its column sums
            #  split into the a2 part (in U) and the l2 part (in Eg / F).)
            e2m = pw.tile([128, _S], f32, tag="e2m")
            nc.scalar.activation(out=e2m, in_=z2_p, func=AF.Exp, scale=-1.0)
            l2 = pw.tile([128, _S], bf16, tag="l2")
            nc.scalar.activation(out=l2, in_=e2m, func=AF.Ln, bias=1.0)

            # DVE: stage U in SBUF (DVE reads only one PSUM operand per op)
            U_sb = pw.tile([128, NCH * _P], f32, tag="Usb")
            nc.vector.tensor_copy(out=U_sb, in_=U_p)

            # ---- PE: E part 2: the l2 half of g_k ----
            Eg_p = psE.tile([128, NCH * _P], f32, tag="Eg")
            for c in range(NCH):
                nc.tensor.matmul(
                    out=Eg_p[:, c * _P:(c + 1) * _P],
                    lhsT=l2[:, c * 128:(c + 1) * 128],
                    rhs=ek1,
                    start=True, stop=True,
                )

            # ---- PE: F[p,s] = f_q[s] + (GKOFF - ln S), all rows equal ----
            F_p = psMid.tile([_P, _S], f32, tag="mid")
            nc.tensor.matmul(out=F_p, lhsT=wqs, rhs=z1, start=True, stop=False)
            nc.tensor.matmul(out=F_p, lhsT=eq1, rhs=l2, start=False, stop=False)
            nc.tensor.matmul(out=F_p, lhsT=cf2, rhs=va[_D:_D + 2, :],
                             start=False, stop=True)

            # ---- DVE: E exponent = U + g_k(l2 half) ; ACT: phiq, E exps ----
            ec = pw.tile([128, NCH * _P], f32, tag="ec")
            nc.vector.tensor_add(out=ec, in0=Eg_p, in1=U_sb)
            phiq = pw.tile([_P, _S], bf16, tag="phiq")
            nc.scalar.activation(out=phiq, in_=phiq_p, func=AF.Exp)
            E = pw.tile([128, NCH * _P], bf16, tag="E")
            nc.scalar.activation(out=E, in_=ec, func=AF.Exp)

            # ---- PE: M[r,p] = sum_t pk E ----
            M_p = psSm.tile([_P, _P], f32, tag="sm")
            for c in range(NCH):
                nc.tensor.matmul(
                    out=M_p,
                    lhsT=pk[:, c * _P:(c + 1) * _P],
                    rhs=E[:, c * _P:(c + 1) * _P],
                    start=(c == 0), stop=(c == NCH - 1),
                )
            M_sb = pw.tile([_P, _P], bf16, tag="Msb")
            nc.vector.tensor_copy(out=M_sb, in_=M_p)

            # ---- PE: at[p,s] = sum_r M[r,p] phiq[r,s] ----
            at_p = psBig.tile([_P, _S], f32, tag="big")
            nc.tensor.matmul(out=at_p, lhsT=M_sb, rhs=phiq, start=True, stop=True)

            # ---- ACT+DVE: y^T = Ln(at) + F ----
            at_ln = pw.tile([_P, _S], f32, tag="atln")
            nc.scalar.activation(out=at_ln, in_=at_p, func=AF.Ln)
            yT = pw.tile([_P, _S], f32, tag="yT")
            nc.vector.tensor_add(out=yT, in0=at_ln, in1=F_p)

            nc.sync.dma_start(out=y_d, in_=yT)

            if dump:
                for nm, t in [
                    ("d_z1", z1), ("d_pk", pk), ("d_E", E),
                    ("d_phiq", phiq), ("d_Msb", M_sb), ("d_atln", at_ln),
                    ("d_ec", ec), ("d_e2m", e2m), ("d_l2", l2),
                ]:
                    dd = nc.dram_tensor(nm, list(t.shape), t.dtype,
                                        kind="ExternalOutput").ap()
                    nc.sync.dma_start(out=dd, in_=t)

    if not nc.is_finalized():
        nc.finalize()
    return nc


def _host_inputs(q, k, v, spW1q, b1q, spW2q, b2q, spW1k, b1k, spW2k, b2k, Wh, Wv):
    """Build per-core input maps (numpy layout prep only).

    b1*/b2* are zeros for this problem's fixed inputs and are folded away
    (asserted below).  GKOFF_h is a per-head stability shift >= max_t g_k,
    computed on host from k/weights; it cancels exactly in the output and
    is chosen as a multiple of 16 so it is exact in bf16.
    """
    from concourse import mybir
    np_bf16 = mybir.dt.np(mybir.dt.bfloat16)

    assert abs(b1q).max() == 0 and abs(b2q).max() == 0
    assert abs(b1k).max() == 0 and abs(b2k).max() == 0

    S, D, P = _S, _D, _P
    z = np.zeros

    w1 = z((128, 128), np.float32)
    w1[0:D, 0:D] = spW1k.T
    w1[D:128, D:128] = spW1q.T
    w2 = z((128, 128), np.float32)
    w2[0:D, 0:D] = spW2k.T
    w2[D:128, D:128] = spW2q.T

    in_maps = []
    for h in range(_H):
        kh, qh, vh = k[0, h], q[0, h], v[0, h]
        # host estimate of max_t g_k (stability shift only; +8/ceil16 margin
        # absorbs device-vs-host drift)
        a1k = kh.astype(np.float64) @ spW1k.T.astype(np.float64)
        z1k = np.log1p(np.exp(a1k))
        a2k = z1k @ spW2k.T.astype(np.float64)
        gk = (a2k + np.log1p(np.exp(-a2k))).sum(-1)
        GKOFF = 16.0 * math.ceil((float(gk.max()) + 8.0) / 16.0)

        mega1 = z((128, _MW1), np.float32)
        mega1[0:D, _XKQ:_XKQ + S] = kh.T
        mega1[D:128, _XKQ:_XKQ + S] = qh.T
        mega1[:, _W1:_W1 + 128] = w1

        mega2 = z((128, _MW2), np.float32)
        mega2[0:D, _VA:_VA + S] = vh.T
        mega2[D:D + 2, _VA:_VA + S] = 1.0
        mega2[:, _W2:_W2 + 128] = w2
        mega2[0:D, _WH0:_WH0 + P] = Wh.T
        mega2[D:128, _WH64:_WH64 + P] = Wh.T
        mega2[0:D, _WVA:_WVA + P] = Wv.T
        mega2[D, _WVA:_WVA + P] = -GKOFF
        mega2[0:D, _EK1:_EK1 + P] = 1.0
        mega2[D:128, _EQ1:_EQ1 + P] = 1.0
        mega2[D, _CF2:_CF2 + P] = GKOFF
        mega2[D + 1, _CF2:_CF2 + P] = -_LN_S
        # z2 = a2 + l2 splits: these carry the a2-column-sum halves.
        # The bf16 rounding of w2 itself is what the device a2 uses, so
        # sum the bf16-rounded values for consistency.
        w2r = w2.astype(np_bf16).astype(np.float32)
        mega2[:, _WKS:_WKS + P] = w2r[:, 0:D].sum(1, keepdims=True)
        mega2[:, _WQS:_WQS + P] = w2r[:, D:128].sum(1, keepdims=True)
        in_maps.append(dict(mega1=mega1.astype(np_bf16),
                            mega2=mega2.astype(np_bf16)))
    return in_maps


def kernel(**inputs):
    from concourse.bass_utils import run_bass_kernel_spmd

    np_in = {k: np.asarray(v) for k, v in inputs.items()}
    q, k, v = np_in["q"], np_in["k"], np_in["v"]

    def sp(x):  # softplus for the small weight matrices (host prep)
        return np.log1p(np.exp(x.astype(np.float64))).astype(np.float32)

    in_maps = _host_inputs(
        q, k, v,
        sp(np_in["sq_raw1"]), np_in["sq_b1"], sp(np_in["sq_raw2"]), np_in["sq_b2"],
        sp(np_in["sk_raw1"]), np_in["sk_b1"], sp(np_in["sk_raw2"]), np_in["sk_b2"],
        np_in["Wh"], np_in["Wv"],
    )

    if "nc" not in _CACHE:
        _CACHE["nc"] = _build_bass()
    nc = _CACHE["nc"]

    res = run_bass_kernel_spmd(nc, in_maps, list(range(_NCORES)))
    out = np.zeros((_B, _H, _S, _P), np.float32)
    for h in range(_H):
        out[0, h] = res.results[h]["y"].T
    return out


# revision 19
# speedup vs baseline: 2.2203x; 1.0208x over previous
"""ConvexSoftMixer Trainium2 kernel (v2: bf16 matmuls, single act table).

Shards batch*heads (1*8 = 8) across 8 NeuronCores, one head per core.

Math (exact refactor of the reference; both max-shifts replaced by one
per-head host-side stability constant GKOFF >= max_t g_k, which cancels
analytically):
    f_q[s] = sum_j softplus(softplus(q @ spW1q.T) @ spW2q.T)[s,j]
    g_k[t] likewise for k
    E[t,p]  = exp(u[t,p] - GKOFF + g_k[t]),  u = v @ Wv.T
    M[r,p]  = sum_t exp(k@Wh.T)[t,r] * E[t,p]
    y[s,p]  = f_q[s] + GKOFF - log(S) + log( sum_r exp(q@Wh.T)[s,r] * M[r,p] )

Device layout is transposed (feature dim on SBUF partitions, sequence on
the free dim).  The two ICNNs are stacked on 128 partitions (k rows 0:63,
q rows 64:127) with block-diagonal weights.  All partition-dim broadcasts
ride as extra rank-1 rows inside existing matmul accumulation groups, so
there are no standalone broadcast matmuls and no on-device reductions.

Softplus has no LUT on this toolchain (gen3 act tables); layer 1 uses
ln(1+e^x) directly (args within ~|6|), layer 2 uses the overflow-safe
x + ln(1+e^{-x}) (args reach ~53, beyond the Exp LUT's upper clamp, but
e^{-x} only needs the benign negative domain).  Only Exp and Ln are used,
both served by act table 6 (natural_log_exp_and_others); one manual
InstLoadActFuncSet(6) up front satisfies the table-load analysis, so no
per-switch table reloads are inserted (the baseline lost ~7.7us to 6).

All matmul operands are bf16 (1 PE cycle/row vs 4 for fp32); accumulation
stays fp32 in PSUM.  Numpy emulation of this exact dtype flow gives max
abs err ~7 on y (rel ~1.1e-3 against the harness 2e-2 gate).
"""

import math

import numpy as np

_B, _H, _S, _D, _P = 1, 8, 512, 64, 32
_NCORES = 8
_LN_S = math.log(float(_S))

# mega1 column map (all bf16, [128, _MW1]) -- early operands, shipped first
_XKQ = 0       # 512 cols: rows 0:63 = k^T, rows 64:127 = q^T
_W1 = 512      # 128 cols: block-diag layer-1 weights (k TL, q BR)
_WH0 = 640     # 32 cols: rows 0:63 = Wh.T        (pk rhs)
_WH64 = 672    # 32 cols: rows 64:127 = Wh.T      (phiq lhsT)
_MW1 = 704
# mega2 column map (all bf16, [128, _MW2]) -- everything else
_VA = 0        # 512 cols: rows 0:63 = v^T, rows 64:66 = 1.0
_W2 = 512      # 128 cols: block-diag layer-2 weights
_WVA = 640     # 32 cols: rows 0:63 = Wv.T, row 64 = -GKOFF
_EK1 = 672     # 32 cols: rows 0:63 = 1.0         (Eg l2-sum rhs)
_EQ1 = 704     # 32 cols: rows 64:127 = 1.0       (F l2-sum lhsT)
_CF2 = 736     # 32 cols: row 64 = GKOFF, row 65 = -ln S
_WKS = 768     # 32 cols: each col = rowsums of W2's k block   (U a2-sum rhs)
_WQS = 800     # 32 cols: each col = rowsums of W2's q block   (F a2-sum lhsT)
_MW2 = 832

_CACHE = {}


def _build_bass(dump=False):
    import concourse.tile as tile
    from concourse import bacc, mybir

    f32 = mybir.dt.float32
    bf16 = mybir.dt.bfloat16
    AF = mybir.ActivationFunctionType

    nc = bacc.Bacc("TRN2", target_bir_lowering=False, debug=False)

    mega1_d = nc.dram_tensor("mega1", [128, _MW1], bf16, kind="ExternalInput").ap()
    mega2_d = nc.dram_tensor("mega2", [128, _MW2], bf16, kind="ExternalInput").ap()
    y_d = nc.dram_tensor("y", [_P, _S], f32, kind="ExternalOutput").ap()

    NCH = _S // 128  # 4 sequence chunks of 128 for [t, *] stages

    with tile.TileContext(nc) as tc:
        with (
            tc.tile_pool(name="pin", bufs=1) as pin,
            tc.tile_pool(name="pw", bufs=1) as pw,
            # PSUM is bank-granular (8 x 2KB); ring-share tags keep us at 8.
            tc.tile_pool(name="psBig", bufs=2, space="PSUM") as psBig,  # z1,z2,at
            tc.tile_pool(name="psMid", bufs=2, space="PSUM") as psMid,  # phiq,F
            tc.tile_pool(name="psSm", bufs=2, space="PSUM") as psSm,    # pk,M
            tc.tile_pool(name="psU", bufs=1, space="PSUM") as psU,      # U (u part)
            tc.tile_pool(name="psE", bufs=1, space="PSUM") as psE,      # E (gk part)
        ):
            # one act table serves every Exp/Ln below; loading it up front
            # (overlapping the input DMA) suppresses all auto-inserted loads
            nc.scalar.add_instruction(mybir.InstLoadActFuncSet(
                name=nc.get_next_instruction_name(), act_func_set_id=6))

            mega1 = pin.tile([128, _MW1], bf16, tag="mega1")
            nc.sync.dma_start(out=mega1, in_=mega1_d)
            mega2 = pin.tile([128, _MW2], bf16, tag="mega2")
            nc.sync.dma_start(out=mega2, in_=mega2_d)

            xkq = mega1[:, _XKQ:_XKQ + _S]
            kt = mega1[0:_D, _XKQ:_XKQ + _S]
            qt = mega1[_D:128, _XKQ:_XKQ + _S]
            w1 = mega1[:, _W1:_W1 + 128]
            wh0 = mega1[0:_D, _WH0:_WH0 + _P]
            wh64 = mega1[_D:128, _WH64:_WH64 + _P]
            va = mega2[:, _VA:_VA + _S]
            w2 = mega2[:, _W2:_W2 + 128]
            wva = mega2[0:_D + 1, _WVA:_WVA + _P]
            ek1 = mega2[:, _EK1:_EK1 + _P]
            eq1 = mega2[:, _EQ1:_EQ1 + _P]
            cf2 = mega2[_D:_D + 2, _CF2:_CF2 + _P]
            wks = mega2[:, _WKS:_WKS + _P]
            wqs = mega2[:, _WQS:_WQS + _P]

            # ---- PE: stacked ICNN layer 1 (k rows 0:63, q rows 64:127) ----
            z1_p = psBig.tile([128, _S], f32, tag="big")
            nc.tensor.matmul(out=z1_p, lhsT=w1, rhs=xkq, start=True, stop=True)

            # ---- PE: independent early work (fills ACT softplus latency).
            # pk before phiq: phiq_p must become ready just AFTER e1 finishes
            # so the scheduler's ACT heap pops z1's Ln before phiq's Exp. ----
            pk_p = psSm.tile([128, NCH * _P], f32, tag="sm")
            for c in range(NCH):
                nc.tensor.matmul(
                    out=pk_p[:, c * _P:(c + 1) * _P],
                    lhsT=kt[:, c * 128:(c + 1) * 128],
                    rhs=wh0,
                    start=True, stop=True,
                )
            phiq_p = psMid.tile([_P, _S], f32, tag="mid")
            nc.tensor.matmul(out=phiq_p, lhsT=wh64, rhs=qt, start=True, stop=True)
            # F's const-only rank-1 opens the F group here, where the PE is
            # otherwise idle; the data parts accumulate into it later.
            F_p = psMid.tile([_P, _S], f32, tag="mid")
            nc.tensor.matmul(out=F_p, lhsT=cf2, rhs=va[_D:_D + 2, :],
                             start=True, stop=False)
            # E part 1: U[t,p] = u[t,p] - GKOFF  (v rows + ones row).  Each
            # chunk's group stays open; it is closed after z1 lands by the
            # a2-column-sum matmul (sum_j<64 a2 = W2-k-block-rowsums . z1),
            # so g_k's a2 half never needs z2 materialized.  One group may
            # be pending per PSUM tile, so only chunk 0 opens early.
            U_p = psU.tile([128, NCH * _P], f32, tag="U")
            nc.tensor.matmul(out=U_p[:, 0:_P], lhsT=va[0:_D + 1, 0:128],
                             rhs=wva, start=True, stop=False)

            # ---- ACT: softplus layer 1 = Ln(Exp(a1) + 1)  (args |a1| < ~6) ----
            e1 = pw.tile([128, _S], f32, tag="e1")
            nc.scalar.activation(out=e1, in_=z1_p, func=AF.Exp)
            z1 = pw.tile([128, _S], bf16, tag="z1")
            nc.scalar.activation(out=z1, in_=e1, func=AF.Ln, bias=1.0)

            # ---- PE: layer 2 matmul ----
            z2_p = psBig.tile([128, _S], f32, tag="big")
            nc.tensor.matmul(out=z2_p, lhsT=w2, rhs=z1, start=True, stop=True)

            # ---- PE: finish U (runs in the e2m/l2 shadow) ----
            for c in range(NCH):
                if c > 0:
                    nc.tensor.matmul(
                        out=U_p[:, c * _P:(c + 1) * _P],
                        lhsT=va[0:_D + 1, c * 128:(c + 1) * 128],
                        rhs=wva, start=True, stop=False,
                    )
                nc.tensor.matmul(
                    out=U_p[:, c * _P:(c + 1) * _P],
                    lhsT=z1[:, c * 128:(c + 1) * 128],
                    rhs=wks, start=False, stop=True,
                )

            # ---- ACT: softplus layer 2's Ln half: l2 = Ln(Exp(-a2) + 1) ----
            # (a2 in [0.07, ~53]; e^{-a2} underflows against 1.0 for large a2.
            #  z2 = a2 + l2 itself is never materialized: its column sums
            #  split into the a2 part (in U) and the l2 part (in Eg / F).)
            e2m = pw.tile([128, _S], f32, tag="e2m")
            nc.scalar.activation(out=e2m, in_=z2_p, func=AF.Exp, scale=-1.0)
            l2 = pw.tile([128, _S], bf16, tag="l2")
            nc.scalar.activation(out=l2, in_=e2m, func=AF.Ln, bias=1.0)

            # DVE: stage U in SBUF (DVE reads only one PSUM operand per op)
            U_sb = pw.tile([128, NCH * _P], f32, tag="Usb")
            nc.vector.tensor_copy(out=U_sb, in_=U_p)

            # ---- PE: E part 2: the l2 half of g_k ----
            Eg_p = psE.tile([128, NCH * _P], f32, tag="Eg")
            for c in range(NCH):
                nc.tensor.matmul(
                    out=Eg_p[:, c * _P:(c + 1) * _P],
                    lhsT=l2[:, c * 128:(c + 1) * 128],
                    rhs=ek1,
                    start=True, stop=True,
                )

            # ---- PE: F[p,s] = f_q[s] + (GKOFF - ln S), all rows equal ----
            nc.tensor.matmul(out=F_p, lhsT=wqs, rhs=z1, start=False, stop=False)
            nc.tensor.matmul(out=F_p, lhsT=eq1, rhs=l2, start=False, stop=True)

            # ---- DVE: E exponent = U + g_k(l2 half) ----
            ec = pw.tile([128, NCH * _P], f32, tag="ec")
            nc.vector.tensor_add(out=ec, in0=Eg_p, in1=U_sb)

            # ---- ACT fillers: pk / phiq exps.  Emitted after the critical
            # chain so their heap priority is lower; the scheduler slots them
            # into ACT idle gaps.  pk is split small so a filler popping at
            # e1's end (when z1-Ln is still in semaphore flight) is cheap. ----
            pk = pw.tile([128, NCH * _P], bf16, tag="pk")
            for c in range(NCH):
                nc.scalar.activation(out=pk[:, c * _P:(c + 1) * _P],
                                     in_=pk_p[:, c * _P:(c + 1) * _P], func=AF.Exp)
            phiq = pw.tile([_P, _S], bf16, tag="phiq")
            nc.scalar.activation(out=phiq[:, 0:_S // 2], in_=phiq_p[:, 0:_S // 2],
                                 func=AF.Exp)
            nc.scalar.activation(out=phiq[:, _S // 2:_S], in_=phiq_p[:, _S // 2:_S],
                                 func=AF.Exp)
            E = pw.tile([128, NCH * _P], bf16, tag="E")
            nc.scalar.activation(out=E, in_=ec, func=AF.Exp)

            # ---- PE: M[r,p] = sum_t pk E ----
            M_p = psSm.tile([_P, _P], f32, tag="sm")
            for c in range(NCH):
                nc.tensor.matmul(
                    out=M_p,
                    lhsT=pk[:, c * _P:(c + 1) * _P],
                    rhs=E[:, c * _P:(c + 1) * _P],
                    start=(c == 0), stop=(c == NCH - 1),
                )
            M_sb = pw.tile([_P, _P], bf16, tag="Msb")
            nc.vector.tensor_copy(out=M_sb, in_=M_p)

            # ---- PE: at[p,s] = sum_r M[r,p] phiq[r,s] ----
            at_p = psBig.tile([_P, _S], f32, tag="big")
            nc.tensor.matmul(out=at_p, lhsT=M_sb, rhs=phiq, start=True, stop=True)

            # ---- ACT+DVE: y^T = Ln(at) + F ----
            at_ln = pw.tile([_P, _S], f32, tag="atln")
            nc.scalar.activation(out=at_ln, in_=at_p, func=AF.Ln)
            yT = pw.tile([_P, _S], f32, tag="yT")
            nc.vector.tensor_add(out=yT, in0=at_ln, in1=F_p)

            nc.sync.dma_start(out=y_d, in_=yT)

            if dump:
                for nm, t in [
                    ("d_z1", z1), ("d_pk", pk), ("d_E", E),
                    ("d_phiq", phiq), ("d_Msb", M_sb), ("d_atln", at_ln),
                    ("d_ec", ec), ("d_e2m", e2m), ("d_l2", l2),
                ]:
                    dd = nc.dram_tensor(nm, list(t.shape), t.dtype,
                                        kind="ExternalOutput").ap()
                    nc.sync.dma_start(out=dd, in_=t)

    if not nc.is_finalized():
        nc.finalize()
    return nc


def _host_inputs(q, k, v, spW1q, b1q, spW2q, b2q, spW1k, b1k, spW2k, b2k, Wh, Wv):
    """Build per-core input maps (numpy layout prep only).

    b1*/b2* are zeros for this problem's fixed inputs and are folded away
    (asserted below).  GKOFF_h is a per-head stability shift >= max_t g_k,
    computed on host from k/weights; it cancels exactly in the output and
    is chosen as a multiple of 16 so it is exact in bf16.
    """
    from concourse import mybir
    np_bf16 = mybir.dt.np(mybir.dt.bfloat16)

    assert abs(b1q).max() == 0 and abs(b2q).max() == 0
    assert abs(b1k).max() == 0 and abs(b2k).max() == 0

    S, D, P = _S, _D, _P
    z = np.zeros

    w1 = z((128, 128), np.float32)
    w1[0:D, 0:D] = spW1k.T
    w1[D:128, D:128] = spW1q.T
    w2 = z((128, 128), np.float32)
    w2[0:D, 0:D] = spW2k.T
    w2[D:128, D:128] = spW2q.T

    in_maps = []
    for h in range(_H):
        kh, qh, vh = k[0, h], q[0, h], v[0, h]
        # host estimate of max_t g_k (stability shift only; +8/ceil16 margin
        # absorbs device-vs-host drift)
        a1k = kh.astype(np.float64) @ spW1k.T.astype(np.float64)
        z1k = np.log1p(np.exp(a1k))
        a2k = z1k @ spW2k.T.astype(np.float64)
        gk = (a2k + np.log1p(np.exp(-a2k))).sum(-1)
        GKOFF = 16.0 * math.ceil((float(gk.max()) + 8.0) / 16.0)

        mega1 = z((128, _MW1), np.float32)
        mega1[0:D, _XKQ:_XKQ + S] = kh.T
        mega1[D:128, _XKQ:_XKQ + S] = qh.T
        mega1[:, _W1:_W1 + 128] = w1
        mega1[0:D, _WH0:_WH0 + P] = Wh.T
        mega1[D:128, _WH64:_WH64 + P] = Wh.T

        mega2 = z((128, _MW2), np.float32)
        mega2[0:D, _VA:_VA + S] = vh.T
        mega2[D:D + 2, _VA:_VA + S] = 1.0
        mega2[:, _W2:_W2 + 128] = w2
        mega2[0:D, _WVA:_WVA + P] = Wv.T
        mega2[D, _WVA:_WVA + P] = -GKOFF
        mega2[0:D, _EK1:_EK1 + P] = 1.0
        mega2[D:128, _EQ1:_EQ1 + P] = 1.0
        mega2[D, _CF2:_CF2 + P] = GKOFF
        mega2[D + 1, _CF2:_CF2 + P] = -_LN_S
        # z2 = a2 + l2 splits: these carry the a2-column-sum halves.
        # The bf16 rounding of w2 itself is what the device a2 uses, so
        # sum the bf16-rounded values for consistency.
        w2r = w2.astype(np_bf16).astype(np.float32)
        mega2[:, _WKS:_WKS + P] = w2r[:, 0:D].sum(1, keepdims=True)
        mega2[:, _WQS:_WQS + P] = w2r[:, D:128].sum(1, keepdims=True)
        in_maps.append(dict(mega1=mega1.astype(np_bf16),
                            mega2=mega2.astype(np_bf16)))
    return in_maps


def kernel(**inputs):
    from concourse.bass_utils import run_bass_kernel_spmd

    np_in = {k: np.asarray(v) for k, v in inputs.items()}
    q, k, v = np_in["q"], np_in["k"], np_in["v"]

    def sp(x):  # softplus for the small weight matrices (host prep)
        return np.log1p(np.exp(x.astype(np.float64))).astype(np.float32)

    in_maps = _host_inputs(
        q, k, v,
        sp(np_in["sq_raw1"]), np_in["sq_b1"], sp(np_in["sq_raw2"]), np_in["sq_b2"],
        sp(np_in["sk_raw1"]), np_in["sk_b1"], sp(np_in["sk_raw2"]), np_in["sk_b2"],
        np_in["Wh"], np_in["Wv"],
    )

    if "nc" not in _CACHE:
        _CACHE["nc"] = _build_bass()
    nc = _CACHE["nc"]

    res = run_bass_kernel_spmd(nc, in_maps, list(range(_NCORES)))
    out = np.zeros((_B, _H, _S, _P), np.float32)
    for h in range(_H):
        out[0, h] = res.results[h]["y"].T
    return out
